# revision 7
# baseline (speedup 1.0000x reference)
"""Bass/Trainium2 kernel v3 for nn_NormAttention (causal linear attention).

Batch+head-sharded SPMD across 8 NeuronCores, no collectives:
core c owns batch b = c//4 and heads {2*(c%4), 2*(c%4)+1}.  Compared to
the v2 head-only sharding (4096 rows x 1 head per core) this processes
2048 rows x 2 heads per core, which

  - halves the x input DMA (2MB instead of 4MB per core),
  - halves the output partial (one batch's rows only, summed over 4
    cores host-side instead of 8),
  - packs both heads' V projection into one M=128 matmul stream and
    both heads' po into one K=128 Wo matmul per row chunk (v2 ran both
    at half the PE array: M=64 / K=64).

PE work per core ~47k moving columns (~20us at 2.4GHz) in ~190 matmuls
vs v2's ~69k columns in 234.  Small attention matmuls (scores/po/state)
are interleaved with the big N=512 projection/Wo streams to keep the
PE HAM clock gate warm (v2 spent its last 40us throttled at 1.2GHz).

Math per head (device, bf16 matmuls / f32 PSUM), rows chunked at 128:
  qkt_h = relu(x_b @ [Wq_h;Wk_h].T + b)     # [128, rows]: q 0:64, k 64:128
  vt    = (x_b @ [Wv_h0|Wv_h1].T + bv) * (1/|k_row|)   # k-norm folded into V
  kc_h  = qkt_h[64:128] shifted to partitions 0:64 (DMA)
  s     = kc_h.T @ q (per 128-chunk), masked causal     # scores
  kvr_h = DMA-transposes of kc_h / vt_h    # row-major K,V per chunk
  po    = V_rows.T @ at + S_{<c}.T @ q     # [64,2? -> packed 128, rows]
  S_c   = K_rows.T @ V_rows; prefix = bf16 SBUF add chain (both heads)
  out_r = [po_h0;po_h1].T @ Wo2            # [rows,512] bf16 partial

1/|q| and 1/|k| row norms span all 8 heads (split across cores), so they
are computed host-side exactly as in v2; host also sums the 4 per-batch
partials, applies 1/|q| and bo.
"""
import numpy as np
import ml_dtypes

import concourse.bacc as bacc
import concourse.tile as tile
import concourse.mybir as mybir
import concourse.bass_utils as bass_utils

F32 = mybir.dt.float32
BF16 = mybir.dt.bfloat16
BF = ml_dtypes.bfloat16
AF = mybir.ActivationFunctionType
ALU = mybir.AluOpType

B, L, E, H, HD = 2, 2048, 512, 8, 64
N = B * L
NCORES = 8
RPC = 2048              # rows per core (one batch)
KT = 4                  # contraction k-tiles (E // 128)
C = 128                 # attention row-chunk
NCH = RPC // C          # 16 row chunks per core
PCW = 512               # proj chunk width (rows)
NPC = RPC // PCW        # 4 proj chunks
EPS = 1e-12

_cache = {}


def _build():
    nc = bacc.Bacc("TRN2", target_bir_lowering=False, debug=False,
                   num_devices=NCORES)

    xt_d = nc.dram_tensor("xt", [128, NPC, KT, PCW], BF16,
                          kind="ExternalInput").ap()
    wqk_d = nc.dram_tensor("wqk", [128, KT, 2, 128], BF16,
                           kind="ExternalInput").ap()
    wv_d = nc.dram_tensor("wv", [128, KT, 128], BF16,
                          kind="ExternalInput").ap()
    wo2_d = nc.dram_tensor("wo2", [128, E], BF16, kind="ExternalInput").ap()
    bqk_d = nc.dram_tensor("bqk", [128, 2], F32, kind="ExternalInput").ap()
    bvp_d = nc.dram_tensor("bvp", [128, 1], F32, kind="ExternalInput").ap()
    mask2_d = nc.dram_tensor("mask2", [128, 2, C], BF16,
                             kind="ExternalInput").ap()
    rk2_d = nc.dram_tensor("rk2", [128, RPC], BF16,
                           kind="ExternalInput").ap()
    out_d = nc.dram_tensor("out", [RPC, E], BF16, kind="ExternalOutput").ap()

    with tile.TileContext(nc) as tc:
        with (
            tc.tile_pool(name="const", bufs=1) as const,
            tc.tile_pool(name="bigp", bufs=1) as bigp,
            tc.tile_pool(name="xtp", bufs=4) as xtp,
            tc.tile_pool(name="atp", bufs=4) as atp,
            tc.tile_pool(name="otp", bufs=4) as otp,
            tc.tile_pool(name="ssbp", bufs=3) as ssbp,
            tc.tile_pool(name="osbp", bufs=4) as osbp,
            tc.tile_pool(name="pjqk", bufs=2, space="PSUM") as pjqk,
            tc.tile_pool(name="pjv", bufs=1, space="PSUM") as pjv,
            tc.tile_pool(name="ps2", bufs=1, space="PSUM") as ps2,
            tc.tile_pool(name="ppo", bufs=1, space="PSUM") as ppo,
            tc.tile_pool(name="pstp", bufs=1, space="PSUM") as pstp,
            tc.tile_pool(name="pwo", bufs=2, space="PSUM") as pwo,
        ):
            # ---- constants (gpsimd DMA queue; wqk first: proj(0) dep) -----
            wqk_sb = const.tile([128, KT, 2, 128], BF16)
            wv_sb = const.tile([128, KT, 128], BF16)
            wo2_sb = const.tile([128, E], BF16)
            bqk_sb = const.tile([128, 2], F32)
            bvp_sb = const.tile([128, 1], F32)
            mask2_sb = const.tile([128, 2, C], BF16)
            rk2_sb = const.tile([128, RPC], BF16)
            nc.gpsimd.dma_start(wqk_sb[:], wqk_d)
            nc.gpsimd.dma_start(wv_sb[:], wv_d)
            nc.gpsimd.dma_start(bqk_sb[:], bqk_d)
            nc.gpsimd.dma_start(bvp_sb[:], bvp_d)
            nc.gpsimd.dma_start(mask2_sb[:], mask2_d)
            nc.gpsimd.dma_start(rk2_sb[:], rk2_d)
            nc.gpsimd.dma_start(wo2_sb[:], wo2_d)

            # ---- x chunk loads: 0,1 on scalar queue, 2,3 on gpsimd -------
            xtiles = {}
            for pc in range(NPC):
                xtile = xtp.tile([128, KT, PCW], BF16, tag="xt", name="xtile")
                eng = nc.scalar if pc < 2 else nc.gpsimd
                eng.dma_start(xtile[:], xt_d[:, pc, :, :])
                xtiles[pc] = xtile

            # ---- PE warm-up bridge while the first DMAs land -------------
            wsc = const.tile([128, 512], BF16)
            nc.vector.memset(wsc[:], 0.0)
            warm = pwo.tile([128, 512], F32, tag="wps", name="warm")
            NWARM = 10
            for i in range(NWARM):
                nc.tensor.matmul(warm[:], wsc[:, 0:128], wsc[:],
                                 start=(i == 0), stop=(i == NWARM - 1))

            # ---- persistent activations ----------------------------------
            qkt = [bigp.tile([128, RPC], BF16, name=f"qkt{h}")
                   for h in range(2)]
            kc = bigp.tile([64, 2, RPC], BF16)      # k cols, partitions 0:64
            vt = bigp.tile([128, RPC], BF16)        # v~ cols: h0 0:64, h1 64:
            kvr = [bigp.tile([128, NCH, 2 * HD], BF16, name=f"kvr{h}")
                   for h in range(2)]

            ps2t = ps2.tile([128, 4, C], F32, tag="s2", name="s2")
            ppo_t = ppo.tile([128, 4, C], F32, tag="po", name="po")
            pst_t = pstp.tile([HD, 8, HD], F32, tag="st", name="st")
            sp_of = {}          # cl -> [64, 2(head), 64] bf16 state product
            pref = {}           # cl -> [64, 2(head), 64] bf16 S_{<cl}

            def proj(pc):
                xtile = xtiles[pc]
                sl = slice(pc * PCW, (pc + 1) * PCW)
                for h in (0, 1):
                    qk = pjqk.tile([128, PCW], F32, tag="pj", name="qkps")
                    for k in range(KT):
                        nc.tensor.matmul(qk[:], wqk_sb[:, k, h, :],
                                         xtile[:, k, :],
                                         start=(k == 0), stop=(k == KT - 1))
                    nc.scalar.activation(qkt[h][:, sl], qk[:], AF.Relu,
                                         bias=bqk_sb[:, h:h + 1])
                    # k rows 64:128 -> kc partitions 0:64 (same queue as the
                    # dependent transposes below for FIFO chaining)
                    nc.sync.dma_start(kc[:, h, sl], qkt[h][64:128, sl])
                vp = pjv.tile([128, PCW], F32, tag="pjv", name="vps")
                for k in range(KT):
                    nc.tensor.matmul(vp[:], wv_sb[:, k, :], xtile[:, k, :],
                                     start=(k == 0), stop=(k == KT - 1))
                # vt = (v + bv) * (1/|k_row|), both heads aligned
                nc.vector.scalar_tensor_tensor(
                    vt[:, sl], vp[:], bvp_sb[:], rk2_sb[:, sl],
                    op0=ALU.add, op1=ALU.mult)
                r0 = pc * 4
                for h in (0, 1):
                    nc.sync.dma_start_transpose(kvr[h][:, r0:r0 + 4, 0:HD],
                                                kc[:, h, sl])
                    nc.sync.dma_start_transpose(
                        kvr[h][:, r0:r0 + 4, HD:2 * HD],
                        vt[h * HD:(h + 1) * HD, sl])

            def prework(cl):
                # scores for both heads of chunk cl + causal mask (Pool)
                rows = slice(cl * C, (cl + 1) * C)
                s0 = (cl % 2) * 2
                for h in (0, 1):
                    nc.tensor.matmul(ps2t[:, s0 + h, :], kc[:, h, rows],
                                     qkt[h][0:64, rows],
                                     start=True, stop=True)
                at2 = atp.tile([128, 2, C], BF16, name="at2")
                nc.vector.tensor_mul(at2[:], ps2t[:, s0:s0 + 2, :],
                                     mask2_sb[:])
                return at2

            def po_block(cl, at2):
                rows = slice(cl * C, (cl + 1) * C)
                s = cl % 4
                for h in (0, 1):
                    nc.tensor.matmul(ppo_t[h * HD:(h + 1) * HD, s, :],
                                     kvr[h][:, cl, HD:2 * HD], at2[:, h, :],
                                     start=True, stop=(cl == 0))
                    if cl > 0:
                        nc.tensor.matmul(ppo_t[h * HD:(h + 1) * HD, s, :],
                                         pref[cl][:, h, :],
                                         qkt[h][0:64, rows],
                                         start=False, stop=True)
                # po -> SBUF bf16 (ACT; DVE is loaded with mask+states)
                ot = otp.tile([128, C], BF16, name="ot")
                nc.scalar.copy(ot[:], ppo_t[:, s, :])
                return ot

            def states_block(cl):
                s = (2 * cl) % 8
                for h in (0, 1):
                    nc.tensor.matmul(pst_t[:, s + h, :], kvr[h][:, cl, 0:HD],
                                     kvr[h][:, cl, HD:2 * HD],
                                     start=True, stop=True)
                sp = ssbp.tile([HD, 2, HD], BF16, tag="stp", bufs=8,
                               name="stp")
                nc.vector.tensor_copy(sp[:], pst_t[:, s:s + 2, :])
                sp_of[cl] = sp
                nxt = cl + 1
                if nxt >= NCH:
                    return
                if cl == 0:
                    pref[1] = sp
                else:
                    # SBUF-only bf16 add chain: Pool engine (PSUM-free)
                    pf = ssbp.tile([HD, 2, HD], BF16, tag="pref", bufs=8,
                                   name="pref")
                    nc.gpsimd.tensor_add(pf[:], pref[cl][:], sp[:])
                    pref[nxt] = pf

            def wo_block(cl, ot):
                rows = slice(cl * C, (cl + 1) * C)
                pw = pwo.tile([128, E], F32, tag="wps", name="wps")
                nc.tensor.matmul(pw[:], ot[:], wo2_sb[:],
                                 start=True, stop=True)
                ob = osbp.tile([128, E], BF16, tag="osb", name="osb")
                # uneven column split: ACT is lighter-loaded than DVE
                nc.scalar.copy(ob[:, 0:320], pw[:, 0:320])
                nc.vector.tensor_copy(ob[:, 320:E], pw[:, 320:E])
                eng = nc.gpsimd if cl % 2 == 0 else nc.sync
                eng.dma_start(out_d[rows, :], ob[:])

            # ---- pipeline ------------------------------------------------
            # step cl: scores(cl+1) | po(cl) | states(cl) | Wo(cl-1), with
            # proj(2)/proj(3) injected at steps 3/7 (proj 0,1 up front).
            proj(0)
            proj(1)
            at_of = {0: prework(0)}
            ot_of = {}
            for step in range(NCH + 1):
                cl = step
                if cl == 3:
                    proj(2)
                if cl == 7:
                    proj(3)
                if cl + 1 <= NCH - 1:
                    at_of[cl + 1] = prework(cl + 1)
                if cl <= NCH - 1:
                    ot_of[cl] = po_block(cl, at_of.pop(cl))
                    states_block(cl)
                if cl - 1 >= 0:
                    wo_block(cl - 1, ot_of.pop(cl - 1))

    nc.compile()
    return nc


def _get_nc():
    if "nc" not in _cache:
        _cache["nc"] = _build()
    return _cache["nc"]


def _host_norms(xs, W, bias):
    """1/max(||relu(xs @ W.T + bias)||, eps) per row, flat [N] f32."""
    p = np.maximum(xs @ W.T + bias, 0.0)
    nrm = np.maximum(np.sqrt(np.sum(p * p, axis=1)), EPS)
    return (1.0 / nrm).astype(np.float32)


def kernel(query, Wq, bq, Wk, bk, Wv, bv, Wo, bo):
    query = np.asarray(query, dtype=np.float32)
    Wq, bq = np.asarray(Wq, np.float32), np.asarray(bq, np.float32)
    Wk, bk = np.asarray(Wk, np.float32), np.asarray(bk, np.float32)
    Wv, bv = np.asarray(Wv, np.float32), np.asarray(bv, np.float32)
    Wo, bo = np.asarray(Wo, np.float32), np.asarray(bo, np.float32)
    assert query.shape == (B, L, E)

    # x = query.reshape(L, B, E) (torch view), then b-major rows
    xs = np.ascontiguousarray(
        query.reshape(L, B, E).transpose(1, 0, 2)).reshape(N, E)

    rq = _host_norms(xs, Wq, bq)
    rk = _host_norms(xs, Wk, bk)

    # per-batch x tiles: [128, pc, kt, n'] with 4KB contiguous rows
    xt_b = []
    rk2_b = []
    for b in range(B):
        xb = xs[b * L:(b + 1) * L]
        xt_b.append(np.ascontiguousarray(
            xb.T.reshape(KT, 128, NPC, PCW).transpose(1, 2, 0, 3)).astype(BF))
        rk2_b.append(np.ascontiguousarray(np.broadcast_to(
            rk[b * L:(b + 1) * L][None, :], (128, RPC))).astype(BF))

    tri = np.triu(np.ones((C, C), np.float32)).astype(BF)
    mask2 = np.ascontiguousarray(
        np.broadcast_to(tri[:, None, :], (C, 2, C)))

    in_maps = []
    for c in range(NCORES):
        b = c // 4
        h0 = 2 * (c % 4)
        cols0 = slice(HD * h0, HD * (h0 + 1))
        cols1 = slice(HD * (h0 + 1), HD * (h0 + 2))
        wqk = np.empty((128, KT, 2, 128), np.float32)
        bqk = np.empty((128, 2), np.float32)
        for h, cols in enumerate((cols0, cols1)):
            wcat = np.concatenate([Wq[cols].T, Wk[cols].T], axis=1)
            wqk[:, :, h, :] = wcat.reshape(KT, 128, 128).transpose(1, 0, 2)
            bqk[:, h] = np.concatenate([bq[cols], bk[cols]])
        vcat = np.concatenate([Wv[cols0].T, Wv[cols1].T], axis=1)
        m = dict(
            xt=xt_b[b],
            wqk=np.ascontiguousarray(wqk).astype(BF),
            wv=np.ascontiguousarray(
                vcat.reshape(KT, 128, 128).transpose(1, 0, 2)).astype(BF),
            wo2=np.ascontiguousarray(
                np.concatenate([Wo[:, cols0].T, Wo[:, cols1].T],
                               axis=0)).astype(BF),
            bqk=bqk,
            bvp=np.concatenate([bv[cols0], bv[cols1]])[:, None]
                .astype(np.float32),
            mask2=mask2,
            rk2=rk2_b[b],
        )
        in_maps.append(m)

    nc = _get_nc()
    res = bass_utils.run_bass_kernel_spmd(nc, in_maps,
                                          core_ids=list(range(NCORES)))
    total = np.zeros((N, E), np.float32)
    for c in range(NCORES):
        b = c // 4
        total[b * L:(b + 1) * L] += res.results[c]["out"].astype(np.float32)
    total *= rq[:, None]

    out = (total.reshape(B, L, E).transpose(1, 0, 2) + bo).reshape(B, L, E)
    return np.ascontiguousarray(out.astype(np.float32))


# revision 15
# speedup vs baseline: 1.0097x; 1.0097x over previous
"""Bass/Trainium2 kernel v3 for nn_NormAttention (causal linear attention).

Batch+head-sharded SPMD across 8 NeuronCores, no collectives:
core c owns batch b = c//4 and heads {2*(c%4), 2*(c%4)+1}.  Compared to
the v2 head-only sharding (4096 rows x 1 head per core) this processes
2048 rows x 2 heads per core, which

  - halves the x input DMA (2MB instead of 4MB per core),
  - halves the output partial (one batch's rows only, summed over 4
    cores host-side instead of 8),
  - packs both heads' V projection into one M=128 matmul stream and
    both heads' po into one K=128 Wo matmul per row chunk (v2 ran both
    at half the PE array: M=64 / K=64).

PE work per core ~47k moving columns (~20us at 2.4GHz) in ~190 matmuls
vs v2's ~69k columns in 234.  Small attention matmuls (scores/po/state)
are interleaved with the big N=512 projection/Wo streams to keep the
PE HAM clock gate warm (v2 spent its last 40us throttled at 1.2GHz).

Math per head (device, bf16 matmuls / f32 PSUM), rows chunked at 128:
  qkt_h = relu(x_b @ [Wq_h;Wk_h].T + b)     # [128, rows]: q 0:64, k 64:128
  vt    = (x_b @ [Wv_h0|Wv_h1].T + bv) * (1/|k_row|)   # k-norm folded into V
  kc_h  = qkt_h[64:128] shifted to partitions 0:64 (DMA)
  s     = kc_h.T @ q (per 128-chunk), masked causal     # scores
  kvr_h = DMA-transposes of kc_h / vt_h    # row-major K,V per chunk
  po    = V_rows.T @ at + S_{<c}.T @ q     # [64,2? -> packed 128, rows]
  S_c   = K_rows.T @ V_rows; prefix = bf16 SBUF add chain (both heads)
  out_r = [po_h0;po_h1].T @ Wo2            # [rows,512] bf16 partial

1/|q| and 1/|k| row norms span all 8 heads (split across cores), so they
are computed host-side exactly as in v2; host also sums the 4 per-batch
partials, applies 1/|q| and bo.
"""
import numpy as np
import ml_dtypes

import concourse.bacc as bacc
import concourse.tile as tile
import concourse.mybir as mybir
import concourse.bass_utils as bass_utils

F32 = mybir.dt.float32
BF16 = mybir.dt.bfloat16
BF = ml_dtypes.bfloat16
AF = mybir.ActivationFunctionType
ALU = mybir.AluOpType

B, L, E, H, HD = 2, 2048, 512, 8, 64
N = B * L
NCORES = 8
RPC = 2048              # rows per core (one batch)
KT = 4                  # contraction k-tiles (E // 128)
C = 128                 # attention row-chunk
NCH = RPC // C          # 16 row chunks per core
PCW = 512               # proj chunk width (rows)
NPC = RPC // PCW        # 4 proj chunks
EPS = 1e-12

_cache = {}


def _build():
    nc = bacc.Bacc("TRN2", target_bir_lowering=False, debug=False,
                   num_devices=NCORES)

    xt_d = nc.dram_tensor("xt", [128, NPC, KT, PCW], BF16,
                          kind="ExternalInput").ap()
    wqk_d = nc.dram_tensor("wqk", [128, KT, 2, 128], BF16,
                           kind="ExternalInput").ap()
    wv_d = nc.dram_tensor("wv", [128, KT, 128], BF16,
                          kind="ExternalInput").ap()
    wo2_d = nc.dram_tensor("wo2", [128, E], BF16, kind="ExternalInput").ap()
    bqk_d = nc.dram_tensor("bqk", [128, 2], F32, kind="ExternalInput").ap()
    bvp_d = nc.dram_tensor("bvp", [128, 1], F32, kind="ExternalInput").ap()
    mask2_d = nc.dram_tensor("mask2", [128, 2, C], BF16,
                             kind="ExternalInput").ap()
    rk2_d = nc.dram_tensor("rk2", [128, RPC], BF16,
                           kind="ExternalInput").ap()
    out_d = nc.dram_tensor("out", [RPC, E], BF16, kind="ExternalOutput").ap()

    with tile.TileContext(nc) as tc:
        with (
            tc.tile_pool(name="const", bufs=1) as const,
            tc.tile_pool(name="bigp", bufs=1) as bigp,
            tc.tile_pool(name="xtp", bufs=4) as xtp,
            tc.tile_pool(name="atp", bufs=4) as atp,
            tc.tile_pool(name="otp", bufs=4) as otp,
            tc.tile_pool(name="ssbp", bufs=3) as ssbp,
            tc.tile_pool(name="osbp", bufs=4) as osbp,
            tc.tile_pool(name="pjqk", bufs=2, space="PSUM") as pjqk,
            tc.tile_pool(name="pjv", bufs=1, space="PSUM") as pjv,
            tc.tile_pool(name="ps2", bufs=1, space="PSUM") as ps2,
            tc.tile_pool(name="ppo", bufs=1, space="PSUM") as ppo,
            tc.tile_pool(name="pstp", bufs=1, space="PSUM") as pstp,
            tc.tile_pool(name="pwo", bufs=2, space="PSUM") as pwo,
        ):
            # ---- constants (gpsimd DMA queue; wqk first: proj(0) dep) -----
            wqk_sb = const.tile([128, KT, 2, 128], BF16)
            wv_sb = const.tile([128, KT, 128], BF16)
            wo2_sb = const.tile([128, E], BF16)
            bqk_sb = const.tile([128, 2], F32)
            bvp_sb = const.tile([128, 1], F32)
            mask2_sb = const.tile([128, 2, C], BF16)
            rk2_sb = const.tile([128, RPC], BF16)
            nc.gpsimd.dma_start(wqk_sb[:], wqk_d)
            nc.gpsimd.dma_start(wv_sb[:], wv_d)
            nc.gpsimd.dma_start(bqk_sb[:], bqk_d)
            nc.gpsimd.dma_start(bvp_sb[:], bvp_d)
            nc.gpsimd.dma_start(mask2_sb[:], mask2_d)
            nc.gpsimd.dma_start(rk2_sb[:], rk2_d)
            nc.gpsimd.dma_start(wo2_sb[:], wo2_d)

            # ---- x chunk loads: 0,1 on scalar queue, 2,3 on gpsimd -------
            xtiles = {}
            for pc in range(NPC):
                xtile = xtp.tile([128, KT, PCW], BF16, tag="xt", name="xtile")
                eng = nc.scalar if pc < 2 else nc.gpsimd
                eng.dma_start(xtile[:], xt_d[:, pc, :, :])
                xtiles[pc] = xtile

            # ---- PE warm-up bridge while the first DMAs land -------------
            wsc = const.tile([128, 512], BF16)
            nc.vector.memset(wsc[:], 0.0)
            warm = pwo.tile([128, 512], F32, tag="wps", name="warm")
            NWARM = 10
            for i in range(NWARM):
                nc.tensor.matmul(warm[:], wsc[:, 0:128], wsc[:],
                                 start=(i == 0), stop=(i == NWARM - 1))

            # ---- persistent activations ----------------------------------
            # ktv0 = [k_h0 0:64 (DMA shift, also scores lhsT) | v~_h0 64:128]
            # ktv1 = [v~_h1 0:64 | k_h1 64:128 (plain copy)]  (vp packed
            #   [v_h1; v_h0] so both stt writes are partition-aligned)
            # kc1  = k_h1 shifted to partitions 0:64 (scores lhsT for h1)
            qkt = [bigp.tile([128, RPC], BF16, name=f"qkt{h}")
                   for h in range(2)]
            ktv0 = bigp.tile([128, RPC], BF16)
            ktv1 = bigp.tile([128, RPC], BF16)
            kc1 = bigp.tile([64, RPC], BF16)
            # kvr0 cols = [k 0:64 | v 64:128]; kvr1 cols = [v 0:64 | k 64:]
            kvr = [bigp.tile([128, NCH, 2 * HD], BF16, name=f"kvr{h}")
                   for h in range(2)]

            ps2t = ps2.tile([128, 4, C], F32, tag="s2", name="s2")
            ppo_t = ppo.tile([128, 4, C], F32, tag="po", name="po")
            pst_t = pstp.tile([HD, 8, HD], F32, tag="st", name="st")
            sp_of = {}          # cl -> [64, 2(head), 64] bf16 state product
            pref = {}           # cl -> [64, 2(head), 64] bf16 S_{<cl}

            def proj(pc):
                xtile = xtiles[pc]
                sl = slice(pc * PCW, (pc + 1) * PCW)
                for h in (0, 1):
                    qk = pjqk.tile([128, PCW], F32, tag="pj", name="qkps")
                    for k in range(KT):
                        nc.tensor.matmul(qk[:], wqk_sb[:, k, h, :],
                                         xtile[:, k, :],
                                         start=(k == 0), stop=(k == KT - 1))
                    nc.scalar.activation(qkt[h][:, sl], qk[:], AF.Relu,
                                         bias=bqk_sb[:, h:h + 1])
                # k shifts on sync (scores-critical, FIFO before transposes)
                nc.sync.dma_start(ktv0[0:64, sl], qkt[0][64:128, sl])
                nc.sync.dma_start(kc1[:, sl], qkt[1][64:128, sl])
                # plain-aligned k_h1 copy for the transpose input (gpsimd)
                nc.gpsimd.dma_start(ktv1[64:128, sl], qkt[1][64:128, sl])
                vp = pjv.tile([128, PCW], F32, tag="pjv", name="vps")
                for k in range(KT):
                    nc.tensor.matmul(vp[:], wv_sb[:, k, :], xtile[:, k, :],
                                     start=(k == 0), stop=(k == KT - 1))
                # v~ = (v + bv) * (1/|k_row|); vp = [v_h1; v_h0] so both
                # halves land partition-aligned
                nc.vector.scalar_tensor_tensor(
                    ktv1[0:64, sl], vp[0:64, :], bvp_sb[0:64, :],
                    rk2_sb[0:64, sl], op0=ALU.add, op1=ALU.mult)
                nc.vector.scalar_tensor_tensor(
                    ktv0[64:128, sl], vp[64:128, :], bvp_sb[64:128, :],
                    rk2_sb[64:128, sl], op0=ALU.add, op1=ALU.mult)

            def transpose_rows(c0, c1):
                # ktv -> row-major kvr for chunks [c0, c1) (one big xbar DMA
                # per head: batching amortizes the ~1us per-op fixed cost)
                sl = slice(c0 * C, c1 * C)
                nc.sync.dma_start_transpose(kvr[0][:, c0:c1, :], ktv0[:, sl])
                nc.sync.dma_start_transpose(kvr[1][:, c0:c1, :], ktv1[:, sl])

            # per-head column slices of row-major kvr: [k | v] vs [v | k]
            KSL = (slice(0, HD), slice(HD, 2 * HD))
            VSL = (slice(HD, 2 * HD), slice(0, HD))

            def prework(cl):
                # scores for both heads of chunk cl + causal mask (DVE)
                rows = slice(cl * C, (cl + 1) * C)
                s0 = (cl % 2) * 2
                nc.tensor.matmul(ps2t[:, s0, :], ktv0[0:64, rows],
                                 qkt[0][0:64, rows], start=True, stop=True)
                nc.tensor.matmul(ps2t[:, s0 + 1, :], kc1[:, rows],
                                 qkt[1][0:64, rows], start=True, stop=True)
                at2 = atp.tile([128, 2, C], BF16, name="at2")
                nc.vector.tensor_mul(at2[:], ps2t[:, s0:s0 + 2, :],
                                     mask2_sb[:])
                return at2

            def po_block(cl, at2):
                rows = slice(cl * C, (cl + 1) * C)
                s = cl % 4
                for h in (0, 1):
                    nc.tensor.matmul(ppo_t[h * HD:(h + 1) * HD, s, :],
                                     kvr[h][:, cl, VSL[h]], at2[:, h, :],
                                     start=True, stop=(cl == 0))
                    if cl > 0:
                        nc.tensor.matmul(ppo_t[h * HD:(h + 1) * HD, s, :],
                                         pref[cl][:, h, :],
                                         qkt[h][0:64, rows],
                                         start=False, stop=True)
                # po -> SBUF bf16 (ACT; DVE is loaded with mask+states)
                ot = otp.tile([128, C], BF16, name="ot")
                nc.scalar.copy(ot[:], ppo_t[:, s, :])
                return ot

            def states_block(cl):
                s = (2 * cl) % 8
                for h in (0, 1):
                    nc.tensor.matmul(pst_t[:, s + h, :],
                                     kvr[h][:, cl, KSL[h]],
                                     kvr[h][:, cl, VSL[h]],
                                     start=True, stop=True)
                sp = ssbp.tile([HD, 2, HD], BF16, tag="stp", bufs=8,
                               name="stp")
                nc.vector.tensor_copy(sp[:], pst_t[:, s:s + 2, :])
                sp_of[cl] = sp
                nxt = cl + 1
                if nxt >= NCH:
                    return
                if cl == 0:
                    pref[1] = sp
                else:
                    # SBUF-only bf16 add chain: Pool engine (PSUM-free)
                    pf = ssbp.tile([HD, 2, HD], BF16, tag="pref", bufs=8,
                                   name="pref")
                    nc.gpsimd.tensor_add(pf[:], pref[cl][:], sp[:])
                    pref[nxt] = pf

            def wo_block(cl, ot):
                rows = slice(cl * C, (cl + 1) * C)
                pw = pwo.tile([128, E], F32, tag="wps", name="wps")
                nc.tensor.matmul(pw[:], ot[:], wo2_sb[:],
                                 start=True, stop=True)
                ob = osbp.tile([128, E], BF16, tag="osb", name="osb")
                # uneven column split: ACT is lighter-loaded than DVE
                nc.scalar.copy(ob[:, 0:320], pw[:, 0:320])
                nc.vector.tensor_copy(ob[:, 320:E], pw[:, 320:E])
                nc.gpsimd.dma_start(out_d[rows, :], ob[:])

            def filler(n=1):
                # dead N=512 matmuls to keep the PE HAM activity monitor
                # above its throttle threshold (else the clock gate halves
                # the PE clock for the small-matmul attention phase)
                fw = pwo.tile([128, 512], F32, tag="wps", name="fill")
                for i in range(n):
                    nc.tensor.matmul(fw[:], wsc[:, 0:128], wsc[:],
                                     start=(i == 0), stop=(i == n - 1))

            # ---- pipeline ------------------------------------------------
            # step cl: scores(cl+1) | po(cl) | states(cl) | Wo(cl-1), with
            # proj(2)/proj(3) injected at steps 3/7 (proj 0,1 up front).
            proj(0)
            proj(1)
            transpose_rows(0, 8)
            at_of = {0: prework(0)}
            ot_of = {}
            for step in range(NCH + 1):
                cl = step
                if cl == 3:
                    proj(2)
                    transpose_rows(8, 12)
                if cl == 7:
                    proj(3)
                    transpose_rows(12, 16)
                if cl + 1 <= NCH - 1:
                    at_of[cl + 1] = prework(cl + 1)
                if cl <= NCH - 1:
                    ot_of[cl] = po_block(cl, at_of.pop(cl))
                    states_block(cl)
                if cl - 1 >= 0:
                    wo_block(cl - 1, ot_of.pop(cl - 1))
                if cl not in (3, 7) and cl < NCH:
                    filler(2)

    nc.compile()
    return nc


def _get_nc():
    if "nc" not in _cache:
        _cache["nc"] = _build()
    return _cache["nc"]


def _host_norms(xs, W, bias):
    """1/max(||relu(xs @ W.T + bias)||, eps) per row, flat [N] f32."""
    p = np.maximum(xs @ W.T + bias, 0.0)
    nrm = np.maximum(np.sqrt(np.sum(p * p, axis=1)), EPS)
    return (1.0 / nrm).astype(np.float32)


def kernel(query, Wq, bq, Wk, bk, Wv, bv, Wo, bo):
    query = np.asarray(query, dtype=np.float32)
    Wq, bq = np.asarray(Wq, np.float32), np.asarray(bq, np.float32)
    Wk, bk = np.asarray(Wk, np.float32), np.asarray(bk, np.float32)
    Wv, bv = np.asarray(Wv, np.float32), np.asarray(bv, np.float32)
    Wo, bo = np.asarray(Wo, np.float32), np.asarray(bo, np.float32)
    assert query.shape == (B, L, E)

    # x = query.reshape(L, B, E) (torch view), then b-major rows
    xs = np.ascontiguousarray(
        query.reshape(L, B, E).transpose(1, 0, 2)).reshape(N, E)

    rq = _host_norms(xs, Wq, bq)
    rk = _host_norms(xs, Wk, bk)

    # per-batch x tiles: [128, pc, kt, n'] with 4KB contiguous rows
    xt_b = []
    rk2_b = []
    for b in range(B):
        xb = xs[b * L:(b + 1) * L]
        xt_b.append(np.ascontiguousarray(
            xb.T.reshape(KT, 128, NPC, PCW).transpose(1, 2, 0, 3)).astype(BF))
        rk2_b.append(np.ascontiguousarray(np.broadcast_to(
            rk[b * L:(b + 1) * L][None, :], (128, RPC))).astype(BF))

    tri = np.triu(np.ones((C, C), np.float32)).astype(BF)
    mask2 = np.ascontiguousarray(
        np.broadcast_to(tri[:, None, :], (C, 2, C)))

    in_maps = []
    for c in range(NCORES):
        b = c // 4
        h0 = 2 * (c % 4)
        cols0 = slice(HD * h0, HD * (h0 + 1))
        cols1 = slice(HD * (h0 + 1), HD * (h0 + 2))
        wqk = np.empty((128, KT, 2, 128), np.float32)
        bqk = np.empty((128, 2), np.float32)
        for h, cols in enumerate((cols0, cols1)):
            wcat = np.concatenate([Wq[cols].T, Wk[cols].T], axis=1)
            wqk[:, :, h, :] = wcat.reshape(KT, 128, 128).transpose(1, 0, 2)
            bqk[:, h] = np.concatenate([bq[cols], bk[cols]])
        # vp psum layout is [v_h1 (0:64) | v_h0 (64:128)] — see ktv comments
        vcat = np.concatenate([Wv[cols1].T, Wv[cols0].T], axis=1)
        m = dict(
            xt=xt_b[b],
            wqk=np.ascontiguousarray(wqk).astype(BF),
            wv=np.ascontiguousarray(
                vcat.reshape(KT, 128, 128).transpose(1, 0, 2)).astype(BF),
            wo2=np.ascontiguousarray(
                np.concatenate([Wo[:, cols0].T, Wo[:, cols1].T],
                               axis=0)).astype(BF),
            bqk=bqk,
            bvp=np.concatenate([bv[cols1], bv[cols0]])[:, None]
                .astype(np.float32),
            mask2=mask2,
            rk2=rk2_b[b],
        )
        in_maps.append(m)

    nc = _get_nc()
    res = bass_utils.run_bass_kernel_spmd(nc, in_maps,
                                          core_ids=list(range(NCORES)))
    total = np.zeros((N, E), np.float32)
    for c in range(NCORES):
        b = c // 4
        total[b * L:(b + 1) * L] += res.results[c]["out"].astype(np.float32)
    total *= rq[:, None]

    out = (total.reshape(B, L, E).transpose(1, 0, 2) + bo).reshape(B, L, E)
    return np.ascontiguousarray(out.astype(np.float32))


# revision 16
# speedup vs baseline: 1.0385x; 1.0286x over previous
"""Bass/Trainium2 kernel v5 for nn_NormAttention (causal linear attention).

Batch+head-sharded SPMD across 8 NeuronCores, no collectives:
core c owns batch b = c//4 and heads {2*(c%4), 2*(c%4)+1} (2048 rows,
2 heads per core: half the x traffic / half the output partial of the
v2 head-only split, and V-proj / Wo run with the full 128-wide array).

Execution shape (per core, rows chunked at C=128, 16 chunks):
  - 4 projection chunks of 512 rows; per chunk 12 N=512 matmuls
    (q|k per head, packed v) — big streaming work for the PE.
  - per row-chunk: 2 score matmuls (N=128, K=64), masked on DVE; po =
    V_rows.T @ at + S_prefix.T @ q (4 matmuls N=128); states K.T@V~
    (2 matmuls N=64); Wo [128,512] (N=512, K=128 both heads packed).
  - row-major K/V tiles come from 6 batched xbar DMA-transposes
    ([128, 512-1024] each; the ~1us/op fixed cost made 16 small ones a
    serial bottleneck in v3).
  - scores and states+prefix run 2 steps ahead of po so the
    states->DVE copy->Pool add->PE inter-matmul cross-engine chain is
    never on the critical path.
  - 2 dead N=512 "filler" matmuls per step keep the PE HAM activity
    window above the clock-gate threshold (otherwise the attention
    phase runs at 1.2GHz instead of 2.4GHz).
  - DMA queues: sync = k shifts + transposes; scalar = x0/x1 loads;
    gpsimd = const blob + x2/x3 + paired output stores.

1/|q| and 1/|k| span all 8 heads (they live on different cores), so
they are computed host-side (as in the graded v2 baseline); 1/|k| is
folded into V on device, 1/|q| + bo into the host-side unshard sum of
the 4 per-batch partials.
"""
import numpy as np
import ml_dtypes

import concourse.bacc as bacc
import concourse.tile as tile
import concourse.mybir as mybir
import concourse.bass_utils as bass_utils

F32 = mybir.dt.float32
BF16 = mybir.dt.bfloat16
BF = ml_dtypes.bfloat16
AF = mybir.ActivationFunctionType
ALU = mybir.AluOpType

B, L, E, H, HD = 2, 2048, 512, 8, 64
N = B * L
NCORES = 8
RPC = 2048              # rows per core (one batch)
KT = 4                  # contraction k-tiles (E // 128)
C = 128                 # attention row-chunk
NCH = RPC // C          # 16 row chunks per core
PCW = 512               # proj chunk width (rows)
NPC = RPC // PCW        # 4 proj chunks
EPS = 1e-12

# const blob column offsets (bf16): wqk | wv | wo2 | mask2 | rk2
OW_QK, OW_V, OW_O, OW_M, OW_R = 0, 1024, 1536, 2048, 2304
CBLOB_W = 2304 + RPC

_cache = {}


def _build():
    nc = bacc.Bacc("TRN2", target_bir_lowering=False, debug=False,
                   num_devices=NCORES)

    xt_d = nc.dram_tensor("xt", [128, NPC, KT, PCW], BF16,
                          kind="ExternalInput").ap()
    cb_d = nc.dram_tensor("cb", [128, CBLOB_W], BF16,
                          kind="ExternalInput").ap()
    bias_d = nc.dram_tensor("bias", [128, 3], F32,
                            kind="ExternalInput").ap()
    out_d = nc.dram_tensor("out", [RPC, E], BF16, kind="ExternalOutput").ap()

    with tile.TileContext(nc) as tc:
        with (
            tc.tile_pool(name="const", bufs=1) as const,
            tc.tile_pool(name="bigp", bufs=1) as bigp,
            tc.tile_pool(name="xtp", bufs=4) as xtp,
            tc.tile_pool(name="atp", bufs=4) as atp,
            tc.tile_pool(name="otp", bufs=4) as otp,
            tc.tile_pool(name="ssbp", bufs=3) as ssbp,
            tc.tile_pool(name="osbp", bufs=3) as osbp,
            tc.tile_pool(name="pjqk", bufs=2, space="PSUM") as pjqk,
            tc.tile_pool(name="pjv", bufs=1, space="PSUM") as pjv,
            tc.tile_pool(name="ps2", bufs=1, space="PSUM") as ps2,
            tc.tile_pool(name="ppo", bufs=1, space="PSUM") as ppo,
            tc.tile_pool(name="pstp", bufs=1, space="PSUM") as pstp,
            tc.tile_pool(name="pwo", bufs=2, space="PSUM") as pwo,
        ):
            # ---- constants: one bf16 blob + one f32 bias DMA -------------
            cblob = const.tile([128, CBLOB_W], BF16)
            bias_sb = const.tile([128, 3], F32)
            nc.gpsimd.dma_start(cblob[:], cb_d)
            nc.gpsimd.dma_start(bias_sb[:], bias_d)

            def w_qk(k, h):
                o = OW_QK + k * 256 + h * 128
                return cblob[:, o:o + 128]

            def w_v(k):
                o = OW_V + k * 128
                return cblob[:, o:o + 128]

            wo2_sb = cblob[:, OW_O:OW_O + E]
            mask2_sb = cblob[:, OW_M:OW_M + 256].rearrange(
                "p (h c) -> p h c", h=2)
            rk2_sb = cblob[:, OW_R:OW_R + RPC]
            bqk_sb = bias_sb[:, 0:2]
            bvp_sb = bias_sb[:, 2:3]

            # ---- x chunk loads: 0,1 on scalar queue, 2,3 on gpsimd -------
            xtiles = {}
            for pc in range(NPC):
                xtile = xtp.tile([128, KT, PCW], BF16, tag="xt", name="xtile")
                eng = nc.scalar if pc < 2 else nc.gpsimd
                eng.dma_start(xtile[:], xt_d[:, pc, :, :])
                xtiles[pc] = xtile

            # ---- PE warm-up bridge while the first DMAs land -------------
            wsc = const.tile([128, 512], BF16)
            nc.vector.memset(wsc[:], 0.0)
            warm = pwo.tile([128, 512], F32, tag="wps", name="warm")
            NWARM = 10
            for i in range(NWARM):
                nc.tensor.matmul(warm[:], wsc[:, 0:128], wsc[:],
                                 start=(i == 0), stop=(i == NWARM - 1))

            # ---- persistent activations ----------------------------------
            # ktv0 = [k_h0 0:64 (DMA shift, also scores lhsT) | v~_h0 64:]
            # ktv1 = [v~_h1 0:64 | k_h1 64:128 (plain copy)]  (vp packed
            #   [v_h1; v_h0] so both stt writes are partition-aligned)
            # kc1  = k_h1 shifted to partitions 0:64 (scores lhsT for h1)
            qkt = [bigp.tile([128, RPC], BF16, name=f"qkt{h}")
                   for h in range(2)]
            ktv0 = bigp.tile([128, RPC], BF16)
            ktv1 = bigp.tile([128, RPC], BF16)
            kc1 = bigp.tile([64, RPC], BF16)
            # kvr0 cols = [k 0:64 | v 64:128]; kvr1 cols = [v 0:64 | k 64:]
            kvr = [bigp.tile([128, NCH, 2 * HD], BF16, name=f"kvr{h}")
                   for h in range(2)]

            ps2t = ps2.tile([128, 4, C], F32, tag="s2", name="s2")
            ppo_t = ppo.tile([128, 4, C], F32, tag="po", name="po")
            pst_t = pstp.tile([HD, 8, HD], F32, tag="st", name="st")
            sp_of = {}          # cl -> [64, 2(head), 64] bf16 state product
            pref = {}           # cl -> [64, 2(head), 64] bf16 S_{<cl}

            def proj(pc):
                xtile = xtiles[pc]
                sl = slice(pc * PCW, (pc + 1) * PCW)
                for h in (0, 1):
                    qk = pjqk.tile([128, PCW], F32, tag="pj", name="qkps")
                    for k in range(KT):
                        nc.tensor.matmul(qk[:], w_qk(k, h), xtile[:, k, :],
                                         start=(k == 0), stop=(k == KT - 1))
                    nc.scalar.activation(qkt[h][:, sl], qk[:], AF.Relu,
                                         bias=bqk_sb[:, h:h + 1])
                # k shifts on sync (scores-critical, FIFO before transposes)
                nc.sync.dma_start(ktv0[0:64, sl], qkt[0][64:128, sl])
                nc.sync.dma_start(kc1[:, sl], qkt[1][64:128, sl])
                # plain-aligned k_h1 copy for the transpose input
                nc.sync.dma_start(ktv1[64:128, sl], qkt[1][64:128, sl])
                vp = pjv.tile([128, PCW], F32, tag="pjv", name="vps")
                for k in range(KT):
                    nc.tensor.matmul(vp[:], w_v(k), xtile[:, k, :],
                                     start=(k == 0), stop=(k == KT - 1))
                # v~ = (v + bv) * (1/|k_row|); vp = [v_h1; v_h0] so both
                # halves land partition-aligned
                nc.vector.scalar_tensor_tensor(
                    ktv1[0:64, sl], vp[0:64, :], bvp_sb[0:64, :],
                    rk2_sb[0:64, sl], op0=ALU.add, op1=ALU.mult)
                nc.vector.scalar_tensor_tensor(
                    ktv0[64:128, sl], vp[64:128, :], bvp_sb[64:128, :],
                    rk2_sb[64:128, sl], op0=ALU.add, op1=ALU.mult)

            def transpose_rows(c0, c1):
                # ktv -> row-major kvr for chunks [c0, c1) (one big xbar DMA
                # per head: batching amortizes the ~1us per-op fixed cost)
                sl = slice(c0 * C, c1 * C)
                nc.sync.dma_start_transpose(kvr[0][:, c0:c1, :], ktv0[:, sl])
                nc.sync.dma_start_transpose(kvr[1][:, c0:c1, :], ktv1[:, sl])

            # per-head column slices of row-major kvr: [k | v] vs [v | k]
            KSL = (slice(0, HD), slice(HD, 2 * HD))
            VSL = (slice(HD, 2 * HD), slice(0, HD))

            def prework(cl):
                # scores for both heads of chunk cl + causal mask (DVE)
                rows = slice(cl * C, (cl + 1) * C)
                s0 = (cl % 2) * 2
                nc.tensor.matmul(ps2t[:, s0, :], ktv0[0:64, rows],
                                 qkt[0][0:64, rows], start=True, stop=True)
                nc.tensor.matmul(ps2t[:, s0 + 1, :], kc1[:, rows],
                                 qkt[1][0:64, rows], start=True, stop=True)
                at2 = atp.tile([128, 2, C], BF16, name="at2")
                nc.vector.tensor_mul(at2[:], ps2t[:, s0:s0 + 2, :],
                                     mask2_sb[:])
                return at2

            def states_block(cl):
                s = (2 * cl) % 8
                for h in (0, 1):
                    nc.tensor.matmul(pst_t[:, s + h, :],
                                     kvr[h][:, cl, KSL[h]],
                                     kvr[h][:, cl, VSL[h]],
                                     start=True, stop=True)
                sp = ssbp.tile([HD, 2, HD], BF16, tag="stp", bufs=8,
                               name="stp")
                nc.vector.tensor_copy(sp[:], pst_t[:, s:s + 2, :])
                sp_of[cl] = sp
                nxt = cl + 1
                if nxt >= NCH:
                    return
                if cl == 0:
                    pref[1] = sp
                else:
                    # SBUF-only bf16 add chain: Pool engine (PSUM-free)
                    pf = ssbp.tile([HD, 2, HD], BF16, tag="pref", bufs=8,
                                   name="pref")
                    nc.gpsimd.tensor_add(pf[:], pref[cl][:], sp[:])
                    pref[nxt] = pf

            def po_block(cl, at2):
                rows = slice(cl * C, (cl + 1) * C)
                s = cl % 4
                for h in (0, 1):
                    nc.tensor.matmul(ppo_t[h * HD:(h + 1) * HD, s, :],
                                     kvr[h][:, cl, VSL[h]], at2[:, h, :],
                                     start=True, stop=(cl == 0))
                    if cl > 0:
                        nc.tensor.matmul(ppo_t[h * HD:(h + 1) * HD, s, :],
                                         pref[cl][:, h, :],
                                         qkt[h][0:64, rows],
                                         start=False, stop=True)
                # po -> SBUF bf16 (ACT; DVE is loaded with mask+states)
                ot = otp.tile([128, C], BF16, name="ot")
                nc.scalar.copy(ot[:], ppo_t[:, s, :])
                return ot

            ob_cur = {}

            def wo_block(cl, ot):
                pw = pwo.tile([128, E], F32, tag="wps", name="wps")
                nc.tensor.matmul(pw[:], ot[:], wo2_sb[:],
                                 start=True, stop=True)
                if cl % 2 == 0:
                    ob_cur["t"] = osbp.tile([128, 2, E], BF16, tag="osb",
                                            name="osb")
                ob = ob_cur["t"]
                j = cl % 2
                # uneven column split: ACT is lighter-loaded than DVE
                nc.scalar.copy(ob[:, j, 0:384], pw[:, 0:384])
                nc.vector.tensor_copy(ob[:, j, 384:E], pw[:, 384:E])
                if j == 1:
                    dst = out_d[(cl - 1) * C:(cl + 1) * C, :].rearrange(
                        "(j p) e -> p j e", j=2)
                    nc.gpsimd.dma_start(dst, ob[:])

            def filler(n):
                # dead N=512 matmuls to keep the PE HAM activity monitor
                # above its throttle threshold (else the clock gate halves
                # the PE clock for the small-matmul attention phase)
                fw = pwo.tile([128, 512], F32, tag="wps", name="fill")
                for i in range(n):
                    nc.tensor.matmul(fw[:], wsc[:, 0:128], wsc[:],
                                     start=(i == 0), stop=(i == n - 1))

            # ---- pipeline ------------------------------------------------
            # step cl: po(cl) | Wo(cl-1) | scores(cl+2) | states(cl+2);
            # scores/states run 2 steps ahead so their cross-engine
            # consumers (mask, state-copy, prefix-add) are off the PE
            # critical path.  proj(2)/proj(3) injected at steps 3/7.
            proj(0)
            proj(1)
            transpose_rows(0, 8)
            at_of = {0: prework(0), 1: prework(1)}
            states_block(0)
            states_block(1)
            ot_of = {}
            for cl in range(NCH + 1):
                if cl == 3:
                    proj(2)
                    transpose_rows(8, 12)
                if cl == 7:
                    proj(3)
                    transpose_rows(12, 16)
                if cl not in (3, 7) and cl < NCH:
                    filler(2)
                if cl <= NCH - 1:
                    ot_of[cl] = po_block(cl, at_of.pop(cl))
                if cl - 1 >= 0:
                    wo_block(cl - 1, ot_of.pop(cl - 1))
                if cl + 2 <= NCH - 1:
                    at_of[cl + 2] = prework(cl + 2)
                    states_block(cl + 2)

    nc.compile()
    return nc


def _get_nc():
    if "nc" not in _cache:
        _cache["nc"] = _build()
    return _cache["nc"]


def _host_norms(xs, W, bias):
    """1/max(||relu(xs @ W.T + bias)||, eps) per row, flat [N] f32."""
    p = np.maximum(xs @ W.T + bias, 0.0)
    nrm = np.maximum(np.sqrt(np.sum(p * p, axis=1)), EPS)
    return (1.0 / nrm).astype(np.float32)


def kernel(query, Wq, bq, Wk, bk, Wv, bv, Wo, bo):
    query = np.asarray(query, dtype=np.float32)
    Wq, bq = np.asarray(Wq, np.float32), np.asarray(bq, np.float32)
    Wk, bk = np.asarray(Wk, np.float32), np.asarray(bk, np.float32)
    Wv, bv = np.asarray(Wv, np.float32), np.asarray(bv, np.float32)
    Wo, bo = np.asarray(Wo, np.float32), np.asarray(bo, np.float32)
    assert query.shape == (B, L, E)

    # x = query.reshape(L, B, E) (torch view), then b-major rows
    xs = np.ascontiguousarray(
        query.reshape(L, B, E).transpose(1, 0, 2)).reshape(N, E)

    rq = _host_norms(xs, Wq, bq)
    rk = _host_norms(xs, Wk, bk)

    # per-batch x tiles: [128, pc, kt, n'] with 4KB contiguous rows
    xt_b = []
    rk2_b = []
    for b in range(B):
        xb = xs[b * L:(b + 1) * L]
        xt_b.append(np.ascontiguousarray(
            xb.T.reshape(KT, 128, NPC, PCW).transpose(1, 2, 0, 3)).astype(BF))
        rk2_b.append(np.ascontiguousarray(np.broadcast_to(
            rk[b * L:(b + 1) * L][None, :], (128, RPC))).astype(BF))

    tri = np.triu(np.ones((C, C), np.float32)).astype(BF)
    mask2 = np.ascontiguousarray(
        np.broadcast_to(tri[:, None, :], (C, 2, C))).reshape(C, 2 * C)

    in_maps = []
    for c in range(NCORES):
        b = c // 4
        h0 = 2 * (c % 4)
        cols0 = slice(HD * h0, HD * (h0 + 1))
        cols1 = slice(HD * (h0 + 1), HD * (h0 + 2))
        wqk = np.empty((128, KT, 2, 128), np.float32)
        bqk = np.empty((128, 2), np.float32)
        for h, cols in enumerate((cols0, cols1)):
            wcat = np.concatenate([Wq[cols].T, Wk[cols].T], axis=1)
            wqk[:, :, h, :] = wcat.reshape(KT, 128, 128).transpose(1, 0, 2)
            bqk[:, h] = np.concatenate([bq[cols], bk[cols]])
        # vp psum layout is [v_h1 (0:64) | v_h0 (64:128)] — see ktv comments
        vcat = np.concatenate([Wv[cols1].T, Wv[cols0].T], axis=1)
        wv = vcat.reshape(KT, 128, 128).transpose(1, 0, 2)
        wo2 = np.concatenate([Wo[:, cols0].T, Wo[:, cols1].T], axis=0)
        cb = np.concatenate([
            wqk.reshape(128, KT * 256),
            wv.reshape(128, KT * 128),
            wo2,
            mask2,
            rk2_b[b],
        ], axis=1).astype(BF)
        assert cb.shape == (128, CBLOB_W)
        bias = np.concatenate(
            [bqk, np.concatenate([bv[cols1], bv[cols0]])[:, None]],
            axis=1).astype(np.float32)
        in_maps.append(dict(xt=xt_b[b], cb=cb, bias=bias))

    nc = _get_nc()
    res = bass_utils.run_bass_kernel_spmd(nc, in_maps,
                                          core_ids=list(range(NCORES)))
    total = np.zeros((N, E), np.float32)
    for c in range(NCORES):
        b = c // 4
        total[b * L:(b + 1) * L] += res.results[c]["out"].astype(np.float32)
    total *= rq[:, None]

    out = (total.reshape(B, L, E).transpose(1, 0, 2) + bo).reshape(B, L, E)
    return np.ascontiguousarray(out.astype(np.float32))


# revision 18
# speedup vs baseline: 1.0640x; 1.0245x over previous
"""Bass/Trainium2 kernel v5 for nn_NormAttention (causal linear attention).

Batch+head-sharded SPMD across 8 NeuronCores, no collectives:
core c owns batch b = c//4 and heads {2*(c%4), 2*(c%4)+1} (2048 rows,
2 heads per core: half the x traffic / half the output partial of the
v2 head-only split, and V-proj / Wo run with the full 128-wide array).

Execution shape (per core, rows chunked at C=128, 16 chunks):
  - 4 projection chunks of 512 rows; per chunk 12 N=512 matmuls
    (q|k per head, packed v) — big streaming work for the PE.
  - per row-chunk: 2 score matmuls (N=128, K=64), masked on DVE; po =
    V_rows.T @ at + S_prefix.T @ q (4 matmuls N=128); states K.T@V~
    (2 matmuls N=64); Wo [128,512] (N=512, K=128 both heads packed).
  - row-major K/V tiles come from 6 batched xbar DMA-transposes
    ([128, 512-1024] each; the ~1us/op fixed cost made 16 small ones a
    serial bottleneck in v3).
  - scores and states+prefix run 2 steps ahead of po so the
    states->DVE copy->Pool add->PE inter-matmul cross-engine chain is
    never on the critical path.
  - 2 dead N=512 "filler" matmuls per step keep the PE HAM activity
    window above the clock-gate threshold (otherwise the attention
    phase runs at 1.2GHz instead of 2.4GHz).
  - DMA queues: sync = k shifts + transposes; scalar = x0/x1 loads;
    gpsimd = const blob + x2/x3 + paired output stores.

1/|q| and 1/|k| span all 8 heads (they live on different cores), so
they are computed host-side (as in the graded v2 baseline); 1/|k| is
folded into V on device, 1/|q| + bo into the host-side unshard sum of
the 4 per-batch partials.
"""
import numpy as np
import ml_dtypes

import concourse.bacc as bacc
import concourse.tile as tile
import concourse.mybir as mybir
import concourse.bass_utils as bass_utils

F32 = mybir.dt.float32
BF16 = mybir.dt.bfloat16
BF = ml_dtypes.bfloat16
AF = mybir.ActivationFunctionType
ALU = mybir.AluOpType

B, L, E, H, HD = 2, 2048, 512, 8, 64
N = B * L
NCORES = 8
RPC = 2048              # rows per core (one batch)
KT = 4                  # contraction k-tiles (E // 128)
C = 128                 # attention row-chunk
NCH = RPC // C          # 16 row chunks per core
PCW = 512               # proj chunk width (rows)
NPC = RPC // PCW        # 4 proj chunks
EPS = 1e-12

# const blob column offsets (bf16): wqk | wv | wo2 | mask2 | rk2
OW_QK, OW_V, OW_O, OW_M, OW_R = 0, 1024, 1536, 2048, 2304
CBLOB_W = 2304 + RPC

_cache = {}


def _build():
    nc = bacc.Bacc("TRN2", target_bir_lowering=False, debug=False,
                   num_devices=NCORES)

    xt_d = nc.dram_tensor("xt", [128, NPC, KT, PCW], BF16,
                          kind="ExternalInput").ap()
    cb_d = nc.dram_tensor("cb", [128, CBLOB_W], BF16,
                          kind="ExternalInput").ap()
    bias_d = nc.dram_tensor("bias", [128, 3], F32,
                            kind="ExternalInput").ap()
    out_d = nc.dram_tensor("out", [RPC, E], BF16, kind="ExternalOutput").ap()

    with tile.TileContext(nc) as tc:
        with (
            tc.tile_pool(name="const", bufs=1) as const,
            tc.tile_pool(name="bigp", bufs=1) as bigp,
            tc.tile_pool(name="xtp", bufs=4) as xtp,
            tc.tile_pool(name="atp", bufs=4) as atp,
            tc.tile_pool(name="otp", bufs=4) as otp,
            tc.tile_pool(name="ssbp", bufs=3) as ssbp,
            tc.tile_pool(name="osbp", bufs=3) as osbp,
            tc.tile_pool(name="pjqk", bufs=2, space="PSUM") as pjqk,
            tc.tile_pool(name="pjv", bufs=1, space="PSUM") as pjv,
            tc.tile_pool(name="ps2", bufs=1, space="PSUM") as ps2,
            tc.tile_pool(name="ppo", bufs=1, space="PSUM") as ppo,
            tc.tile_pool(name="pstp", bufs=1, space="PSUM") as pstp,
            tc.tile_pool(name="pwo", bufs=2, space="PSUM") as pwo,
        ):
            # ---- constants: wqk first (proj(0) dep), then the rest -------
            cblob = const.tile([128, CBLOB_W], BF16)
            bias_sb = const.tile([128, 3], F32)
            nc.gpsimd.dma_start(cblob[:, 0:OW_V], cb_d[:, 0:OW_V])
            nc.gpsimd.dma_start(bias_sb[:], bias_d)
            nc.gpsimd.dma_start(cblob[:, OW_V:], cb_d[:, OW_V:])

            def w_qk(k, h):
                o = OW_QK + k * 256 + h * 128
                return cblob[:, o:o + 128]

            def w_v(k):
                o = OW_V + k * 128
                return cblob[:, o:o + 128]

            wo2_sb = cblob[:, OW_O:OW_O + E]
            mask2_sb = cblob[:, OW_M:OW_M + 256].rearrange(
                "p (h c) -> p h c", h=2)
            rk2_sb = cblob[:, OW_R:OW_R + RPC]
            bqk_sb = bias_sb[:, 0:2]
            bvp_sb = bias_sb[:, 2:3]

            # ---- x chunk loads: 0,1 on scalar queue, 2,3 on gpsimd -------
            xtiles = {}
            for pc in range(NPC):
                xtile = xtp.tile([128, KT, PCW], BF16, tag="xt", name="xtile")
                eng = nc.scalar if pc < 2 else nc.gpsimd
                eng.dma_start(xtile[:], xt_d[:, pc, :, :])
                xtiles[pc] = xtile

            # ---- PE warm-up bridge while the first DMAs land -------------
            wsc = const.tile([128, 512], BF16)
            nc.vector.memset(wsc[:], 0.0)
            warm = pwo.tile([128, 512], F32, tag="wps", name="warm")
            NWARM = 10
            for i in range(NWARM):
                nc.tensor.matmul(warm[:], wsc[:, 0:128], wsc[:],
                                 start=(i == 0), stop=(i == NWARM - 1))

            # ---- persistent activations ----------------------------------
            # ktv0 = [k_h0 0:64 (DMA shift, also scores lhsT) | v~_h0 64:]
            # ktv1 = [v~_h1 0:64 | k_h1 64:128 (plain copy)]  (vp packed
            #   [v_h1; v_h0] so both stt writes are partition-aligned)
            # kc1  = k_h1 shifted to partitions 0:64 (scores lhsT for h1)
            qkt = [bigp.tile([128, RPC], BF16, name=f"qkt{h}")
                   for h in range(2)]
            ktv0 = bigp.tile([128, RPC], BF16)
            ktv1 = bigp.tile([128, RPC], BF16)
            kc1 = bigp.tile([64, RPC], BF16)
            # kvr0 cols = [k 0:64 | v 64:128]; kvr1 cols = [v 0:64 | k 64:]
            kvr = [bigp.tile([128, NCH, 2 * HD], BF16, name=f"kvr{h}")
                   for h in range(2)]

            ps2t = ps2.tile([128, 4, C], F32, tag="s2", name="s2")
            ppo_t = ppo.tile([128, 4, C], F32, tag="po", name="po")
            pst_t = pstp.tile([HD, 8, HD], F32, tag="st", name="st")
            sp_of = {}          # cl -> [64, 2(head), 64] bf16 state product
            pref = {}           # cl -> [64, 2(head), 64] bf16 S_{<cl}

            def proj(pc):
                xtile = xtiles[pc]
                sl = slice(pc * PCW, (pc + 1) * PCW)
                for h in (0, 1):
                    qk = pjqk.tile([128, PCW], F32, tag="pj", name="qkps")
                    for k in range(KT):
                        nc.tensor.matmul(qk[:], w_qk(k, h), xtile[:, k, :],
                                         start=(k == 0), stop=(k == KT - 1))
                    nc.scalar.activation(qkt[h][:, sl], qk[:], AF.Relu,
                                         bias=bqk_sb[:, h:h + 1])
                # k shifts on sync (scores-critical, FIFO before transposes)
                nc.sync.dma_start(ktv0[0:64, sl], qkt[0][64:128, sl])
                nc.sync.dma_start(kc1[:, sl], qkt[1][64:128, sl])
                # plain-aligned k_h1 copy for the transpose input
                nc.sync.dma_start(ktv1[64:128, sl], qkt[1][64:128, sl])
                vp = pjv.tile([128, PCW], F32, tag="pjv", name="vps")
                for k in range(KT):
                    nc.tensor.matmul(vp[:], w_v(k), xtile[:, k, :],
                                     start=(k == 0), stop=(k == KT - 1))
                # v~ = (v + bv) * (1/|k_row|); vp = [v_h1; v_h0] so both
                # halves land partition-aligned
                nc.vector.scalar_tensor_tensor(
                    ktv1[0:64, sl], vp[0:64, :], bvp_sb[0:64, :],
                    rk2_sb[0:64, sl], op0=ALU.add, op1=ALU.mult)
                nc.vector.scalar_tensor_tensor(
                    ktv0[64:128, sl], vp[64:128, :], bvp_sb[64:128, :],
                    rk2_sb[64:128, sl], op0=ALU.add, op1=ALU.mult)

            def transpose_rows(c0, c1):
                # ktv -> row-major kvr for chunks [c0, c1) (one big xbar DMA
                # per head: batching amortizes the ~1us per-op fixed cost)
                sl = slice(c0 * C, c1 * C)
                nc.sync.dma_start_transpose(kvr[0][:, c0:c1, :], ktv0[:, sl])
                nc.sync.dma_start_transpose(kvr[1][:, c0:c1, :], ktv1[:, sl])

            # per-head column slices of row-major kvr: [k | v] vs [v | k]
            KSL = (slice(0, HD), slice(HD, 2 * HD))
            VSL = (slice(HD, 2 * HD), slice(0, HD))

            def prework(cl):
                # scores for both heads of chunk cl + causal mask (DVE)
                rows = slice(cl * C, (cl + 1) * C)
                s0 = (cl % 2) * 2
                nc.tensor.matmul(ps2t[:, s0, :], ktv0[0:64, rows],
                                 qkt[0][0:64, rows], start=True, stop=True)
                nc.tensor.matmul(ps2t[:, s0 + 1, :], kc1[:, rows],
                                 qkt[1][0:64, rows], start=True, stop=True)
                at2 = atp.tile([128, 2, C], BF16, name="at2")
                nc.vector.tensor_mul(at2[:], ps2t[:, s0:s0 + 2, :],
                                     mask2_sb[:])
                return at2

            def states_block(cl):
                s = (2 * cl) % 8
                for h in (0, 1):
                    nc.tensor.matmul(pst_t[:, s + h, :],
                                     kvr[h][:, cl, KSL[h]],
                                     kvr[h][:, cl, VSL[h]],
                                     start=True, stop=True)
                sp = ssbp.tile([HD, 2, HD], BF16, tag="stp", bufs=8,
                               name="stp")
                nc.vector.tensor_copy(sp[:], pst_t[:, s:s + 2, :])
                sp_of[cl] = sp
                nxt = cl + 1
                if nxt >= NCH:
                    return
                if cl == 0:
                    pref[1] = sp
                else:
                    # SBUF-only bf16 add chain: Pool engine (PSUM-free)
                    pf = ssbp.tile([HD, 2, HD], BF16, tag="pref", bufs=8,
                                   name="pref")
                    nc.gpsimd.tensor_add(pf[:], pref[cl][:], sp[:])
                    pref[nxt] = pf

            def po_block(cl, at2):
                rows = slice(cl * C, (cl + 1) * C)
                s = cl % 4
                for h in (0, 1):
                    nc.tensor.matmul(ppo_t[h * HD:(h + 1) * HD, s, :],
                                     kvr[h][:, cl, VSL[h]], at2[:, h, :],
                                     start=True, stop=(cl == 0))
                    if cl > 0:
                        nc.tensor.matmul(ppo_t[h * HD:(h + 1) * HD, s, :],
                                         pref[cl][:, h, :],
                                         qkt[h][0:64, rows],
                                         start=False, stop=True)
                # po -> SBUF bf16 (ACT; DVE is loaded with mask+states)
                ot = otp.tile([128, C], BF16, name="ot")
                nc.scalar.copy(ot[:], ppo_t[:, s, :])
                return ot

            ob_cur = {}

            def wo_block(cl, ot):
                pw = pwo.tile([128, E], F32, tag="wps", name="wps")
                nc.tensor.matmul(pw[:], ot[:], wo2_sb[:],
                                 start=True, stop=True)
                if cl % 2 == 0:
                    ob_cur["t"] = osbp.tile([128, 2, E], BF16, tag="osb",
                                            name="osb")
                ob = ob_cur["t"]
                j = cl % 2
                # uneven column split: ACT is lighter-loaded than DVE
                nc.scalar.copy(ob[:, j, 0:384], pw[:, 0:384])
                nc.vector.tensor_copy(ob[:, j, 384:E], pw[:, 384:E])
                if j == 1:
                    dst = out_d[(cl - 1) * C:(cl + 1) * C, :].rearrange(
                        "(j p) e -> p j e", j=2)
                    nc.gpsimd.dma_start(dst, ob[:])

            def filler(n):
                # dead N=512 matmuls to keep the PE HAM activity monitor
                # above its throttle threshold (else the clock gate halves
                # the PE clock for the small-matmul attention phase)
                fw = pwo.tile([128, 512], F32, tag="wps", name="fill")
                for i in range(n):
                    nc.tensor.matmul(fw[:], wsc[:, 0:128], wsc[:],
                                     start=(i == 0), stop=(i == n - 1))

            # ---- pipeline ------------------------------------------------
            # step cl: scores(cl+2) | po(cl) | Wo(cl-1) | states(cl+2);
            # scores/states run 2 steps ahead so their cross-engine
            # consumers (mask, state-copy, prefix-add) are off the PE
            # critical path.  proj(2)/proj(3) injected early (steps 2/5)
            # so the sync-queue shift->transpose convoy resolves well
            # before states needs the row-major tiles.  states is emitted
            # last in each step: it is the most likely to wait on a fresh
            # transpose, and the in-order PE queue would stall everything
            # behind it.
            proj(0)
            proj(1)
            transpose_rows(0, 8)
            at_of = {0: prework(0), 1: prework(1)}
            states_block(0)
            states_block(1)
            ot_of = {}
            for cl in range(NCH + 1):
                if cl == 2:
                    proj(2)
                    transpose_rows(8, 12)
                if cl == 5:
                    proj(3)
                    transpose_rows(12, 16)
                if cl not in (2, 5) and cl < NCH:
                    filler(2)
                if cl + 2 <= NCH - 1:
                    at_of[cl + 2] = prework(cl + 2)
                if cl <= NCH - 1:
                    ot_of[cl] = po_block(cl, at_of.pop(cl))
                if cl - 1 >= 0:
                    wo_block(cl - 1, ot_of.pop(cl - 1))
                if cl + 2 <= NCH - 1:
                    states_block(cl + 2)

    nc.compile()
    return nc


def _get_nc():
    if "nc" not in _cache:
        _cache["nc"] = _build()
    return _cache["nc"]


def _host_norms(xs, W, bias):
    """1/max(||relu(xs @ W.T + bias)||, eps) per row, flat [N] f32."""
    p = np.maximum(xs @ W.T + bias, 0.0)
    nrm = np.maximum(np.sqrt(np.sum(p * p, axis=1)), EPS)
    return (1.0 / nrm).astype(np.float32)


def kernel(query, Wq, bq, Wk, bk, Wv, bv, Wo, bo):
    query = np.asarray(query, dtype=np.float32)
    Wq, bq = np.asarray(Wq, np.float32), np.asarray(bq, np.float32)
    Wk, bk = np.asarray(Wk, np.float32), np.asarray(bk, np.float32)
    Wv, bv = np.asarray(Wv, np.float32), np.asarray(bv, np.float32)
    Wo, bo = np.asarray(Wo, np.float32), np.asarray(bo, np.float32)
    assert query.shape == (B, L, E)

    # x = query.reshape(L, B, E) (torch view), then b-major rows
    xs = np.ascontiguousarray(
        query.reshape(L, B, E).transpose(1, 0, 2)).reshape(N, E)

    rq = _host_norms(xs, Wq, bq)
    rk = _host_norms(xs, Wk, bk)

    # per-batch x tiles: [128, pc, kt, n'] with 4KB contiguous rows
    xt_b = []
    rk2_b = []
    for b in range(B):
        xb = xs[b * L:(b + 1) * L]
        xt_b.append(np.ascontiguousarray(
            xb.T.reshape(KT, 128, NPC, PCW).transpose(1, 2, 0, 3)).astype(BF))
        rk2_b.append(np.ascontiguousarray(np.broadcast_to(
            rk[b * L:(b + 1) * L][None, :], (128, RPC))).astype(BF))

    tri = np.triu(np.ones((C, C), np.float32)).astype(BF)
    mask2 = np.ascontiguousarray(
        np.broadcast_to(tri[:, None, :], (C, 2, C))).reshape(C, 2 * C)

    in_maps = []
    for c in range(NCORES):
        b = c // 4
        h0 = 2 * (c % 4)
        cols0 = slice(HD * h0, HD * (h0 + 1))
        cols1 = slice(HD * (h0 + 1), HD * (h0 + 2))
        wqk = np.empty((128, KT, 2, 128), np.float32)
        bqk = np.empty((128, 2), np.float32)
        for h, cols in enumerate((cols0, cols1)):
            wcat = np.concatenate([Wq[cols].T, Wk[cols].T], axis=1)
            wqk[:, :, h, :] = wcat.reshape(KT, 128, 128).transpose(1, 0, 2)
            bqk[:, h] = np.concatenate([bq[cols], bk[cols]])
        # vp psum layout is [v_h1 (0:64) | v_h0 (64:128)] — see ktv comments
        vcat = np.concatenate([Wv[cols1].T, Wv[cols0].T], axis=1)
        wv = vcat.reshape(KT, 128, 128).transpose(1, 0, 2)
        wo2 = np.concatenate([Wo[:, cols0].T, Wo[:, cols1].T], axis=0)
        cb = np.concatenate([
            wqk.reshape(128, KT * 256),
            wv.reshape(128, KT * 128),
            wo2,
            mask2,
            rk2_b[b],
        ], axis=1).astype(BF)
        assert cb.shape == (128, CBLOB_W)
        bias = np.concatenate(
            [bqk, np.concatenate([bv[cols1], bv[cols0]])[:, None]],
            axis=1).astype(np.float32)
        in_maps.append(dict(xt=xt_b[b], cb=cb, bias=bias))

    nc = _get_nc()
    res = bass_utils.run_bass_kernel_spmd(nc, in_maps,
                                          core_ids=list(range(NCORES)))
    total = np.zeros((N, E), np.float32)
    for c in range(NCORES):
        b = c // 4
        total[b * L:(b + 1) * L] += res.results[c]["out"].astype(np.float32)
    total *= rq[:, None]

    out = (total.reshape(B, L, E).transpose(1, 0, 2) + bo).reshape(B, L, E)
    return np.ascontiguousarray(out.astype(np.float32))


# revision 21
# speedup vs baseline: 1.1214x; 1.0539x over previous
"""Bass/Trainium2 kernel v5 for nn_NormAttention (causal linear attention).

Batch+head-sharded SPMD across 8 NeuronCores, no collectives:
core c owns batch b = c//4 and heads {2*(c%4), 2*(c%4)+1} (2048 rows,
2 heads per core: half the x traffic / half the output partial of the
v2 head-only split, and V-proj / Wo run with the full 128-wide array).

Execution shape (per core, rows chunked at C=128, 16 chunks):
  - 4 projection chunks of 512 rows; per chunk 12 N=512 matmuls
    (q|k per head, packed v) — big streaming work for the PE.
  - per row-chunk: 2 score matmuls (N=128, K=64), masked on DVE; po =
    V_rows.T @ at + S_prefix.T @ q (4 matmuls N=128); states K.T@V~
    (2 matmuls N=64); Wo [128,512] (N=512, K=128 both heads packed).
  - row-major K/V tiles come from 6 batched xbar DMA-transposes
    ([128, 512-1024] each; the ~1us/op fixed cost made 16 small ones a
    serial bottleneck in v3).
  - scores and states+prefix run 2 steps ahead of po so the
    states->DVE copy->Pool add->PE inter-matmul cross-engine chain is
    never on the critical path.
  - 2 dead N=512 "filler" matmuls per step keep the PE HAM activity
    window above the clock-gate threshold (otherwise the attention
    phase runs at 1.2GHz instead of 2.4GHz).
  - DMA queues: sync = k shifts + transposes; scalar = x0/x1 loads;
    gpsimd = const blob + x2/x3 + paired output stores.

1/|q| and 1/|k| span all 8 heads (they live on different cores), so
they are computed host-side (as in the graded v2 baseline); 1/|k| is
folded into V on device, 1/|q| + bo into the host-side unshard sum of
the 4 per-batch partials.
"""
import numpy as np
import ml_dtypes

import concourse.bacc as bacc
import concourse.tile as tile
import concourse.mybir as mybir
import concourse.bass_utils as bass_utils

F32 = mybir.dt.float32
BF16 = mybir.dt.bfloat16
BF = ml_dtypes.bfloat16
AF = mybir.ActivationFunctionType
ALU = mybir.AluOpType

B, L, E, H, HD = 2, 2048, 512, 8, 64
N = B * L
NCORES = 8
RPC = 2048              # rows per core (one batch)
KT = 4                  # contraction k-tiles (E // 128)
C = 128                 # attention row-chunk
NCH = RPC // C          # 16 row chunks per core
PCW = 512               # proj chunk width (rows)
NPC = RPC // PCW        # 4 proj chunks
EPS = 1e-12

# const blob column offsets (bf16): wqk | wv | wo2 | mask2 | rk2
OW_QK, OW_V, OW_O, OW_M, OW_R = 0, 1024, 1536, 2048, 2304
CBLOB_W = 2304 + RPC

_cache = {}


def _build():
    nc = bacc.Bacc("TRN2", target_bir_lowering=False, debug=False,
                   num_devices=NCORES)

    xt_d = nc.dram_tensor("xt", [128, NPC, KT, PCW], BF16,
                          kind="ExternalInput").ap()
    cb_d = nc.dram_tensor("cb", [128, CBLOB_W], BF16,
                          kind="ExternalInput").ap()
    bias_d = nc.dram_tensor("bias", [128, 3], F32,
                            kind="ExternalInput").ap()
    out_d = nc.dram_tensor("out", [RPC, E], BF16, kind="ExternalOutput").ap()

    with tile.TileContext(nc) as tc:
        with (
            tc.tile_pool(name="const", bufs=1) as const,
            tc.tile_pool(name="bigp", bufs=1) as bigp,
            tc.tile_pool(name="xtp", bufs=4) as xtp,
            tc.tile_pool(name="atp", bufs=4) as atp,
            tc.tile_pool(name="otp", bufs=4) as otp,
            tc.tile_pool(name="ssbp", bufs=3) as ssbp,
            tc.tile_pool(name="osbp", bufs=3) as osbp,
            tc.tile_pool(name="pjqk", bufs=2, space="PSUM") as pjqk,
            tc.tile_pool(name="pjv", bufs=1, space="PSUM") as pjv,
            tc.tile_pool(name="ps2", bufs=1, space="PSUM") as ps2,
            tc.tile_pool(name="ppo", bufs=1, space="PSUM") as ppo,
            tc.tile_pool(name="pstp", bufs=1, space="PSUM") as pstp,
            tc.tile_pool(name="pwo", bufs=2, space="PSUM") as pwo,
        ):
            # ---- constants: wqk first (proj(0) dep), then the rest -------
            cblob = const.tile([128, CBLOB_W], BF16)
            bias_sb = const.tile([128, 3], F32)
            nc.gpsimd.dma_start(cblob[:, 0:OW_V], cb_d[:, 0:OW_V])
            nc.gpsimd.dma_start(bias_sb[:], bias_d)
            nc.gpsimd.dma_start(cblob[:, OW_V:], cb_d[:, OW_V:])

            def w_qk(k, h):
                o = OW_QK + k * 256 + h * 128
                return cblob[:, o:o + 128]

            def w_v(k):
                o = OW_V + k * 128
                return cblob[:, o:o + 128]

            wo2_sb = cblob[:, OW_O:OW_O + E]
            mask2_sb = cblob[:, OW_M:OW_M + 256].rearrange(
                "p (h c) -> p h c", h=2)
            rk2_sb = cblob[:, OW_R:OW_R + RPC]
            bqk_sb = bias_sb[:, 0:2]
            bvp_sb = bias_sb[:, 2:3]

            # ---- x chunk loads: 0,1 on scalar queue, 2,3 on gpsimd -------
            xtiles = {}
            for pc in range(NPC):
                xtile = xtp.tile([128, KT, PCW], BF16, tag="xt", name="xtile")
                eng = nc.scalar if pc < 3 else nc.gpsimd
                eng.dma_start(xtile[:], xt_d[:, pc, :, :])
                xtiles[pc] = xtile

            # ---- PE warm-up bridge while the first DMAs land -------------
            wsc = const.tile([128, 512], BF16)
            nc.vector.memset(wsc[:], 0.0)
            warm = pwo.tile([128, 512], F32, tag="wps", name="warm")
            NWARM = 10
            for i in range(NWARM):
                nc.tensor.matmul(warm[:], wsc[:, 0:128], wsc[:],
                                 start=(i == 0), stop=(i == NWARM - 1))

            # ---- persistent activations ----------------------------------
            # ktv0 = [k_h0 0:64 (DMA shift, also scores lhsT) | v~_h0 64:]
            # ktv1 = [v~_h1 0:64 | k_h1 64:128 (plain copy)]  (vp packed
            #   [v_h1; v_h0] so both stt writes are partition-aligned)
            # kc1  = k_h1 shifted to partitions 0:64 (scores lhsT for h1)
            qkt = [bigp.tile([128, RPC], BF16, name=f"qkt{h}")
                   for h in range(2)]
            ktv0 = bigp.tile([128, RPC], BF16)
            ktv1 = bigp.tile([128, RPC], BF16)
            kc1 = bigp.tile([64, RPC], BF16)
            # kvr0 cols = [k 0:64 | v 64:128]; kvr1 cols = [v 0:64 | k 64:]
            kvr = [bigp.tile([128, NCH, 2 * HD], BF16, name=f"kvr{h}")
                   for h in range(2)]

            ps2t = ps2.tile([128, 4, C], F32, tag="s2", name="s2")
            ppo_t = ppo.tile([128, 4, C], F32, tag="po", name="po")
            pst_t = pstp.tile([HD, 8, HD], F32, tag="st", name="st")
            sp_of = {}          # cl -> [64, 2(head), 64] bf16 state product
            pref = {}           # cl -> [64, 2(head), 64] bf16 S_{<cl}

            def proj(pc):
                xtile = xtiles[pc]
                sl = slice(pc * PCW, (pc + 1) * PCW)
                for h in (0, 1):
                    qk = pjqk.tile([128, PCW], F32, tag="pj", name="qkps")
                    for k in range(KT):
                        nc.tensor.matmul(qk[:], w_qk(k, h), xtile[:, k, :],
                                         start=(k == 0), stop=(k == KT - 1))
                    nc.scalar.activation(qkt[h][:, sl], qk[:], AF.Relu,
                                         bias=bqk_sb[:, h:h + 1])
                # k shifts on sync (scores-critical, FIFO before transposes)
                nc.sync.dma_start(ktv0[0:64, sl], qkt[0][64:128, sl])
                nc.sync.dma_start(kc1[:, sl], qkt[1][64:128, sl])
                # plain-aligned k_h1 copy for the transpose input
                nc.sync.dma_start(ktv1[64:128, sl], qkt[1][64:128, sl])
                vp = pjv.tile([128, PCW], F32, tag="pjv", name="vps")
                for k in range(KT):
                    nc.tensor.matmul(vp[:], w_v(k), xtile[:, k, :],
                                     start=(k == 0), stop=(k == KT - 1))
                # v~ = (v + bv) * (1/|k_row|); vp = [v_h1; v_h0] so both
                # halves land partition-aligned
                nc.vector.scalar_tensor_tensor(
                    ktv1[0:64, sl], vp[0:64, :], bvp_sb[0:64, :],
                    rk2_sb[0:64, sl], op0=ALU.add, op1=ALU.mult)
                nc.vector.scalar_tensor_tensor(
                    ktv0[64:128, sl], vp[64:128, :], bvp_sb[64:128, :],
                    rk2_sb[64:128, sl], op0=ALU.add, op1=ALU.mult)

            def transpose_rows(c0, c1):
                # ktv -> row-major kvr for chunks [c0, c1) (one big xbar DMA
                # per head: batching amortizes the ~1us per-op fixed cost)
                sl = slice(c0 * C, c1 * C)
                nc.sync.dma_start_transpose(kvr[0][:, c0:c1, :], ktv0[:, sl])
                nc.sync.dma_start_transpose(kvr[1][:, c0:c1, :], ktv1[:, sl])

            # per-head column slices of row-major kvr: [k | v] vs [v | k]
            KSL = (slice(0, HD), slice(HD, 2 * HD))
            VSL = (slice(HD, 2 * HD), slice(0, HD))

            def prework(cl):
                # scores for both heads of chunk cl + causal mask (DVE)
                rows = slice(cl * C, (cl + 1) * C)
                s0 = (cl % 2) * 2
                nc.tensor.matmul(ps2t[:, s0, :], ktv0[0:64, rows],
                                 qkt[0][0:64, rows], start=True, stop=True)
                nc.tensor.matmul(ps2t[:, s0 + 1, :], kc1[:, rows],
                                 qkt[1][0:64, rows], start=True, stop=True)
                at2 = atp.tile([128, 2, C], BF16, name="at2")
                nc.vector.tensor_mul(at2[:], ps2t[:, s0:s0 + 2, :],
                                     mask2_sb[:])
                return at2

            def states_block(cl):
                s = (2 * cl) % 8
                for h in (0, 1):
                    nc.tensor.matmul(pst_t[:, s + h, :],
                                     kvr[h][:, cl, KSL[h]],
                                     kvr[h][:, cl, VSL[h]],
                                     start=True, stop=True)
                nxt = cl + 1
                if nxt >= NCH:
                    return
                # prefix fused with the PSUM drain: one DVE op per chunk
                # (pref[cl+1] = pref[cl] + S_cl), no Pool / no extra copy
                pf = ssbp.tile([HD, 2, HD], BF16, tag="pref", bufs=8,
                               name="pref")
                if cl == 0:
                    nc.vector.tensor_copy(pf[:], pst_t[:, s:s + 2, :])
                else:
                    nc.vector.tensor_add(pf[:], pref[cl][:],
                                         pst_t[:, s:s + 2, :])
                pref[nxt] = pf

            def po_block(cl, at2):
                rows = slice(cl * C, (cl + 1) * C)
                s = cl % 4
                for h in (0, 1):
                    nc.tensor.matmul(ppo_t[h * HD:(h + 1) * HD, s, :],
                                     kvr[h][:, cl, VSL[h]], at2[:, h, :],
                                     start=True, stop=(cl == 0))
                    if cl > 0:
                        nc.tensor.matmul(ppo_t[h * HD:(h + 1) * HD, s, :],
                                         pref[cl][:, h, :],
                                         qkt[h][0:64, rows],
                                         start=False, stop=True)
                # po -> SBUF bf16 (ACT; DVE is loaded with mask+states)
                ot = otp.tile([128, C], BF16, name="ot")
                nc.scalar.copy(ot[:], ppo_t[:, s, :])
                return ot

            ob_cur = {}

            def wo_block(cl, ot):
                pw = pwo.tile([128, E], F32, tag="wps", name="wps")
                nc.tensor.matmul(pw[:], ot[:], wo2_sb[:],
                                 start=True, stop=True)
                if cl % 2 == 0:
                    ob_cur["t"] = osbp.tile([128, 2, E], BF16, tag="osb",
                                            name="osb")
                ob = ob_cur["t"]
                j = cl % 2
                # uneven column split: ACT is lighter-loaded than DVE
                nc.scalar.copy(ob[:, j, 0:384], pw[:, 0:384])
                nc.vector.tensor_copy(ob[:, j, 384:E], pw[:, 384:E])
                if j == 1:
                    dst = out_d[(cl - 1) * C:(cl + 1) * C, :].rearrange(
                        "(j p) e -> p j e", j=2)
                    nc.gpsimd.dma_start(dst, ob[:])

            def filler(n):
                # dead N=512 matmuls to keep the PE HAM activity monitor
                # above its throttle threshold (else the clock gate halves
                # the PE clock for the small-matmul attention phase)
                fw = pwo.tile([128, 512], F32, tag="wps", name="fill")
                for i in range(n):
                    nc.tensor.matmul(fw[:], wsc[:, 0:128], wsc[:],
                                     start=(i == 0), stop=(i == n - 1))

            # ---- pipeline ------------------------------------------------
            # Projections are front-loaded (0-2 before the loop, 3 at step
            # 1): the PE streams ~14us of dense N=512 matmuls while the
            # sync-queue shift->transpose convoy resolves, so every
            # row-major kvr tile is ready long before states needs it.
            # step cl: scores(cl+2) | po(cl) | Wo(cl-1) | states(cl+2);
            # the 2-step lookahead keeps the cross-engine consumers (DVE
            # mask, fused prefix-add) off the in-order PE queue's critical
            # path.
            proj(0)
            proj(1)
            transpose_rows(0, 8)
            proj(2)
            transpose_rows(8, 12)
            at_of = {0: prework(0), 1: prework(1)}
            states_block(0)
            states_block(1)
            ot_of = {}
            for cl in range(NCH + 1):
                if cl == 1:
                    proj(3)
                    transpose_rows(12, 16)
                elif cl < NCH:
                    filler(1)
                if cl + 2 <= NCH - 1:
                    at_of[cl + 2] = prework(cl + 2)
                if cl <= NCH - 1:
                    ot_of[cl] = po_block(cl, at_of.pop(cl))
                if cl - 1 >= 0:
                    wo_block(cl - 1, ot_of.pop(cl - 1))
                if cl + 2 <= NCH - 1:
                    states_block(cl + 2)

    nc.compile()
    return nc


def _get_nc():
    if "nc" not in _cache:
        _cache["nc"] = _build()
    return _cache["nc"]


def _host_norms(xs, W, bias):
    """1/max(||relu(xs @ W.T + bias)||, eps) per row, flat [N] f32."""
    p = np.maximum(xs @ W.T + bias, 0.0)
    nrm = np.maximum(np.sqrt(np.sum(p * p, axis=1)), EPS)
    return (1.0 / nrm).astype(np.float32)


def kernel(query, Wq, bq, Wk, bk, Wv, bv, Wo, bo):
    query = np.asarray(query, dtype=np.float32)
    Wq, bq = np.asarray(Wq, np.float32), np.asarray(bq, np.float32)
    Wk, bk = np.asarray(Wk, np.float32), np.asarray(bk, np.float32)
    Wv, bv = np.asarray(Wv, np.float32), np.asarray(bv, np.float32)
    Wo, bo = np.asarray(Wo, np.float32), np.asarray(bo, np.float32)
    assert query.shape == (B, L, E)

    # x = query.reshape(L, B, E) (torch view), then b-major rows
    xs = np.ascontiguousarray(
        query.reshape(L, B, E).transpose(1, 0, 2)).reshape(N, E)

    rq = _host_norms(xs, Wq, bq)
    rk = _host_norms(xs, Wk, bk)

    # per-batch x tiles: [128, pc, kt, n'] with 4KB contiguous rows
    xt_b = []
    rk2_b = []
    for b in range(B):
        xb = xs[b * L:(b + 1) * L]
        xt_b.append(np.ascontiguousarray(
            xb.T.reshape(KT, 128, NPC, PCW).transpose(1, 2, 0, 3)).astype(BF))
        rk2_b.append(np.ascontiguousarray(np.broadcast_to(
            rk[b * L:(b + 1) * L][None, :], (128, RPC))).astype(BF))

    tri = np.triu(np.ones((C, C), np.float32)).astype(BF)
    mask2 = np.ascontiguousarray(
        np.broadcast_to(tri[:, None, :], (C, 2, C))).reshape(C, 2 * C)

    in_maps = []
    for c in range(NCORES):
        b = c // 4
        h0 = 2 * (c % 4)
        cols0 = slice(HD * h0, HD * (h0 + 1))
        cols1 = slice(HD * (h0 + 1), HD * (h0 + 2))
        wqk = np.empty((128, KT, 2, 128), np.float32)
        bqk = np.empty((128, 2), np.float32)
        for h, cols in enumerate((cols0, cols1)):
            wcat = np.concatenate([Wq[cols].T, Wk[cols].T], axis=1)
            wqk[:, :, h, :] = wcat.reshape(KT, 128, 128).transpose(1, 0, 2)
            bqk[:, h] = np.concatenate([bq[cols], bk[cols]])
        # vp psum layout is [v_h1 (0:64) | v_h0 (64:128)] — see ktv comments
        vcat = np.concatenate([Wv[cols1].T, Wv[cols0].T], axis=1)
        wv = vcat.reshape(KT, 128, 128).transpose(1, 0, 2)
        wo2 = np.concatenate([Wo[:, cols0].T, Wo[:, cols1].T], axis=0)
        cb = np.concatenate([
            wqk.reshape(128, KT * 256),
            wv.reshape(128, KT * 128),
            wo2,
            mask2,
            rk2_b[b],
        ], axis=1).astype(BF)
        assert cb.shape == (128, CBLOB_W)
        bias = np.concatenate(
            [bqk, np.concatenate([bv[cols1], bv[cols0]])[:, None]],
            axis=1).astype(np.float32)
        in_maps.append(dict(xt=xt_b[b], cb=cb, bias=bias))

    nc = _get_nc()
    res = bass_utils.run_bass_kernel_spmd(nc, in_maps,
                                          core_ids=list(range(NCORES)))
    total = np.zeros((N, E), np.float32)
    for c in range(NCORES):
        b = c // 4
        total[b * L:(b + 1) * L] += res.results[c]["out"].astype(np.float32)
    total *= rq[:, None]

    out = (total.reshape(B, L, E).transpose(1, 0, 2) + bo).reshape(B, L, E)
    return np.ascontiguousarray(out.astype(np.float32))
